# revision 38
# baseline (speedup 1.0000x reference)
"""AttentionHead kernel for 8 TRN2 NeuronCores.

Reference computation (B=4, S=2048, D=1024, dk=dv=64):
    q = query @ Wq + bq ; k = key @ Wk + bk ; v = value @ Wv + bv
    out = softmax(q @ k.T / 8) @ v

Sharding: core i handles batch b = i//2, query seq-half h = i%2 (1024 rows).
Each core reads its q half + the full k/v of its batch (k/v are read by two
cores; no collectives — the pair-AllGather measured ~35us end-to-end, far
more than the 6us of extra DMA it saves).

Layout strategy: activations are HOST-PRE-TRANSPOSED and PRE-SWIZZLED to
partition-major bf16 so that (a) the contraction dim D is on partitions with
no on-device transposes, (b) every DMA is 128 fat contiguous descriptors
(HWDGE issue time was 2.3us/load at 1024 descriptors, ~0.4us at 128), and
(c) bf16 halves HBM traffic and is the compute precision anyway.  All loads
ride the Sync HWDGE ring in FIFO order = HBM stream order:
  kT  [P, 8c*2048] full key     (4 MiB, 2 half-loads) -> kt_pad
  qT  [P, 8c*1024] query half   (2 MiB)               -> qt_pad
  vT  [P, 8t*8c*256] full value (4 MiB, 8 block-loads) -> vt -> v rows
Weights are host-pre-swizzled [128, 8*64]; Wq/bq pre-scaled by 1/8.

Phase 2 (scores -> exp -> attn@v) streams over key chunks, trailing the v
load.  qt/kt are zero-PADDED to 128 partitions so every matmul contracts
K=128: the PE Hardware Activity Monitor un-throttles (1.2 -> 2.4 GHz) based
on array activity, and half-height matmuls were observed stuck at K=4/8.

Engine budget: Scalar does the exps (one [128,1024] exp per key chunk,
spanning two PSUM banks) plus small DMA issues.  DVE does projection
copy-outs (+bias, f32 PSUM -> bf16 SBUF) and the po copy-out.  PE does
projections, scores/attn@v, and the 16 v transposes.

Softmax notes:
  - bk cancels in softmax and is dropped.
  - bv is folded into the v projection: attn rows sum to 1 after the
    normalization, so attn @ (v + bv) = attn@v + bv.
  - max-subtraction skipped: scores have std ~1/3, exp() is safe in f32.
  - denominator comes free from a ones-column appended to v; the final
    division happens on the HOST during unshard (output is [65, seq]:
    64 unnormalized rows + the denominator row).
"""

import os
import sys

if "/opt/trn_rl_repo" not in sys.path:
    sys.path.insert(0, "/opt/trn_rl_repo")

import ml_dtypes
import numpy as np

import concourse.bass as bass
import concourse.mybir as mybir
import concourse.tile as tile
from concourse import bacc
from concourse.bass_utils import run_bass_kernel_spmd
from concourse.masks import make_identity

N_CORES = 8
B, S, D, DK = 4, 2048, 1024, 64
S_LOC = S // 2          # per-core q rows
P = 128
F32 = mybir.dt.float32
BF16 = mybir.dt.bfloat16
NP_BF16 = ml_dtypes.bfloat16

D_CHUNKS = D // P        # 8 contraction chunks
QTILE = 512              # matmul free-dim tile (one PSUM bank of f32)
N_QT = S_LOC // QTILE    # 2 query tiles
K_CHUNKS = S // P        # 16 key chunks in phase 2
VW = DK + 1              # v plus ones-column
VBLK = 256               # v load block (columns): fine-grained streaming
N_VBLK = S // VBLK       # so phase 2 trails the v load chunk by chunk


def build_program():
    nc = bacc.Bacc("TRN2", target_bir_lowering=False, debug=False,
                   num_devices=N_CORES)

    qT = nc.dram_tensor("qT", [P, D_CHUNKS * S_LOC], BF16, kind="ExternalInput")
    kT = nc.dram_tensor("kT", [P, D_CHUNKS * S], BF16, kind="ExternalInput")
    vT = nc.dram_tensor("vT", [P, N_VBLK * D_CHUNKS * VBLK], BF16,
                        kind="ExternalInput")
    # k/q/v weights concatenated into one tensor -> one DMA
    w3 = nc.dram_tensor("W3", [P, 3 * D_CHUNKS * DK], BF16,
                        kind="ExternalInput")
    bq = nc.dram_tensor("bq", [DK, 1], F32, kind="ExternalInput")
    bv = nc.dram_tensor("bv", [DK, 1], F32, kind="ExternalInput")
    out = nc.dram_tensor("out", [VW, S_LOC], F32, kind="ExternalOutput")

    from contextlib import ExitStack

    with tile.TileContext(nc) as tc, ExitStack() as ctx:
        consts = ctx.enter_context(tc.tile_pool(name="consts", bufs=1))
        sbuf = ctx.enter_context(tc.tile_pool(name="sbuf", bufs=1))
        expp = ctx.enter_context(tc.tile_pool(name="expp", bufs=4))
        outp = ctx.enter_context(tc.tile_pool(name="outp", bufs=2))
        # proj + transpose psums share 2 slots; pss 2x2 banks; po 2 banks
        scr = ctx.enter_context(tc.tile_pool(name="scr", bufs=2, space="PSUM"))
        pssp = ctx.enter_context(tc.tile_pool(name="pssp", bufs=2, space="PSUM"))
        accp = ctx.enter_context(tc.tile_pool(name="accp", bufs=1, space="PSUM"))

        # ---- big SBUF tiles --------------------------------------------
        actq = sbuf.tile([P, D_CHUNKS, S_LOC], BF16, tag="actq")
        actk = sbuf.tile([P, D_CHUNKS, S], BF16, tag="actk")
        actv = sbuf.tile([P, N_VBLK, D_CHUNKS, VBLK], BF16, tag="actv")
        qt_pad = sbuf.tile([P, S_LOC], BF16, tag="qt_pad")
        kt_pad = sbuf.tile([P, S], BF16, tag="kt_pad")
        vt = sbuf.tile([DK, S], BF16, tag="vt")
        v_full = sbuf.tile([P, K_CHUNKS, VW], BF16, tag="v_full")

        # ---- loads (sync HWDGE ring FIFO = HBM stream order) -----------
        # weights first (they gate all projections), then k in two
        # contraction-half loads so half the k projection runs during the
        # load phase instead of on the post-load critical path
        w3_sb = consts.tile([P, 3, D_CHUNKS, DK], BF16, tag="w3")
        nc.sync.dma_start(
            w3_sb[:], w3.rearrange("p (n c k) -> p n c k", n=3, k=DK))
        w_sbs = {nm: w3_sb[:, i] for i, nm in enumerate(("k", "q", "v"))}
        rr_k = kT.rearrange("p (c s) -> p c s", s=S)
        CH = D_CHUNKS // 2
        nc.sync.dma_start(actk[:, :CH], rr_k[:, :CH])
        nc.sync.dma_start(actk[:, CH:], rr_k[:, CH:])
        nc.sync.dma_start(actq[:], qT.rearrange("p (c s) -> p c s", s=S_LOC))
        rr_v = vT.rearrange("p (t c s) -> p t c s", t=N_VBLK, c=D_CHUNKS)
        for t in range(N_VBLK):
            nc.sync.dma_start(actv[:, t], rr_v[:, t])

        # small consts on the scalar HWDGE ring; zero-fill pads on DVE
        bq_sb = consts.tile([DK, 1], F32, tag="bq")
        nc.scalar.dma_start(bq_sb[:], bq[:])
        bv_sb = consts.tile([DK, 1], F32, tag="bv")
        nc.scalar.dma_start(bv_sb[:], bv[:])
        nc.vector.memset(qt_pad[DK:P, :], 0.0)
        nc.vector.memset(kt_pad[DK:P, :], 0.0)
        nc.vector.memset(v_full[:, :, DK:VW], 1.0)
        ident_bf = consts.tile([P, P], BF16)
        make_identity(nc, ident_bf)

        def project(w_sb, rhs_of_c, ps):
            for c in range(D_CHUNKS):
                nc.tensor.matmul(ps[:], w_sb[:, c, :], rhs_of_c(c),
                                 start=(c == 0), stop=(c == D_CHUNKS - 1))

        # ---- key: two-pass projection (c halves arrive separately) -----
        ktA = sbuf.tile([DK, S], F32, tag="ktA")
        for t in range(S // QTILE):
            tcols = slice(t * QTILE, (t + 1) * QTILE)
            ps = scr.tile([DK, QTILE], F32, tag="s", name=f"pka{t}")
            for c in range(CH):
                nc.tensor.matmul(ps[:], w_sbs["k"][:, c, :],
                                 actk[:, c, tcols],
                                 start=(c == 0), stop=(c == CH - 1))
            nc.vector.tensor_copy(ktA[:, tcols], ps[:])
        for t in range(S // QTILE):
            tcols = slice(t * QTILE, (t + 1) * QTILE)
            ps = scr.tile([DK, QTILE], F32, tag="s", name=f"pkb{t}")
            for c in range(CH, D_CHUNKS):
                nc.tensor.matmul(ps[:], w_sbs["k"][:, c, :],
                                 actk[:, c, tcols],
                                 start=(c == CH), stop=(c == D_CHUNKS - 1))
            nc.vector.tensor_add(kt_pad[:DK, tcols], ps[:], ktA[:, tcols])

        # ---- query: project -> qt_pad[:64] (pre-scaled by 1/8) ---------
        for t in range(N_QT):
            tcols = slice(t * QTILE, (t + 1) * QTILE)
            ps = scr.tile([DK, QTILE], F32, tag="s", name=f"psq{t}")
            project(w_sbs["q"], lambda c: actq[:, c, tcols], ps)
            nc.vector.tensor_add(qt_pad[:DK, tcols], ps[:],
                                 bq_sb[:].to_broadcast((DK, QTILE)))

        # ---- value windows + phase 2, interleaved so the PE stream -----
        # alternates [v-proj window u] -> [score/exp/attn chunks 4u..4u+3]
        # and phase 2 trails the v load instead of waiting for all of it.
        po = [accp.tile([VW, N_QT, QTILE], F32, tag="po", name="po0")]

        def phase2_chunk(kc):
            # matmul free dim is ISA-capped at 512, so two mms per step;
            # the exp still covers both PSUM banks in one instruction
            pss = pssp.tile([P, N_QT, QTILE], F32, tag="pss",
                            name=f"pss{kc}")
            for t in range(N_QT):
                nc.tensor.matmul(pss[:, t, :],
                                 kt_pad[:, kc * P:(kc + 1) * P],
                                 qt_pad[:, t * QTILE:(t + 1) * QTILE],
                                 start=True, stop=True)
            e = expp.tile([P, N_QT, QTILE], BF16, tag="exp", name=f"e{kc}")
            nc.scalar.activation(e[:], pss[:],
                                 mybir.ActivationFunctionType.Exp)
            for t in range(N_QT):
                nc.tensor.matmul(po[0][:, t, :], v_full[:, kc, :],
                                 e[:, t, :],
                                 start=(kc == 0), stop=(kc == K_CHUNKS - 1))

        blk_per_w = QTILE // VBLK
        for w in range(S // QTILE):
            wcols = slice(w * QTILE, (w + 1) * QTILE)
            ps = scr.tile([DK, QTILE], F32, tag="s", name=f"psv{w}")
            project(w_sbs["v"],
                    lambda c: actv[:, w * blk_per_w:(w + 1) * blk_per_w, c, :],
                    ps)
            # fold bv into v: attn rows sum to 1 after normalization
            nc.vector.tensor_add(vt[:, wcols], ps[:],
                                 bv_sb[:].to_broadcast((DK, QTILE)))
            for j in range(QTILE // P):
                sb = w * (QTILE // P) + j
                pv = scr.tile([P, DK], BF16, tag="s", name=f"pv{sb}")
                nc.tensor.transpose(pv[:], vt[:, sb * P:(sb + 1) * P],
                                    ident_bf[:DK, :DK])
                nc.vector.tensor_copy(v_full[:, sb, :DK], pv[:])
            for kc in range(w * (QTILE // P), (w + 1) * (QTILE // P)):
                phase2_chunk(kc)

        # ---- epilogue: ship unnormalized out + denominator row ---------
        posb = outp.tile([VW, N_QT, QTILE], F32, tag="posb")
        nc.vector.tensor_copy(posb[:], po[0][:])
        nc.scalar.dma_start(out[:].rearrange("a (t b) -> a t b", t=N_QT),
                            posb[:])

    nc.compile()
    return nc


_CACHED = {}


def _get_program():
    if "nc" not in _CACHED:
        _CACHED["nc"] = build_program()
    return _CACHED["nc"]


def _swizzle_w(w, scale=1.0):
    # [D, DK] -> [P, D_CHUNKS*DK] bf16, partition-major
    w = np.asarray(w, np.float32) * scale
    return np.ascontiguousarray(
        w.reshape(D_CHUNKS, P, DK).transpose(1, 0, 2).reshape(P, -1)
        .astype(NP_BF16))


def _swizzle_act(x):
    # [s, D] -> [P, D_CHUNKS*s] bf16: xT split D=(c p), partition-major
    s = x.shape[0]
    return np.ascontiguousarray(
        x.T.reshape(D_CHUNKS, P, s).transpose(1, 0, 2).reshape(P, -1)
        .astype(NP_BF16))


def _swizzle_v(v):
    # [S, D] -> [P, N_VBLK*D_CHUNKS*VBLK] bf16, v-block-major
    return np.ascontiguousarray(
        v.T.reshape(D_CHUNKS, P, N_VBLK, VBLK).transpose(1, 2, 0, 3)
        .reshape(P, -1).astype(NP_BF16))


def make_in_maps(query, key, value, Wq, bq, Wk, bk, Wv, bv):
    # bk is unused: it only shifts scores by a per-query constant, which
    # cancels in softmax.  Wq/bq absorb the 1/sqrt(dk)=1/8 softmax scale.
    q = np.asarray(query, dtype=np.float32)
    k = np.asarray(key, dtype=np.float32)
    v = np.asarray(value, dtype=np.float32)
    consts = {
        # order matches w_sbs indexing on device: k, q, v
        "W3": np.ascontiguousarray(np.concatenate(
            [_swizzle_w(Wk), _swizzle_w(Wq, 0.125), _swizzle_w(Wv)], axis=1)),
        "bq": np.ascontiguousarray(
            np.asarray(bq, np.float32).reshape(-1, 1) * 0.125),
        "bv": np.ascontiguousarray(np.asarray(bv, np.float32).reshape(-1, 1)),
    }
    kT = [_swizzle_act(k[b]) for b in range(B)]
    vT = [_swizzle_v(v[b]) for b in range(B)]
    in_maps = []
    for i in range(N_CORES):
        b, h = divmod(i, 2)
        sl = slice(h * S_LOC, (h + 1) * S_LOC)
        in_maps.append({
            "qT": _swizzle_act(q[b, sl]),
            "kT": kT[b],
            "vT": vT[b],
            **consts,
        })
    return in_maps


def assemble_output(results):
    out = np.empty((B, S, DK), np.float32)
    for i in range(N_CORES):
        b, h = divmod(i, 2)
        r = results[i]["out"]
        out[b, h * S_LOC:(h + 1) * S_LOC, :] = (r[:DK] / r[DK:VW]).T
    return out


def kernel(query, key, value, Wq, bq, Wk, bk, Wv, bv, **run_kwargs):
    nc = _get_program()
    in_maps = make_in_maps(query, key, value, Wq, bq, Wk, bk, Wv, bv)
    res = run_bass_kernel_spmd(nc, in_maps, core_ids=list(range(N_CORES)),
                               **run_kwargs)
    out = assemble_output(res.results)
    if run_kwargs.get("trace"):
        kernel.last_result = res
    return out
